# revision 3
# baseline (speedup 1.0000x reference)
"""Trainium2 Bass kernel for nn_Bert_69698729280007 (v4).

Data-parallel over batch: core b processes batch row b (2 chunks of 512
tokens through the 4-layer BERT encoder), then an offset-based segment
mean-pool.  No collectives.

v4 changes over v3:
- Embed: bf16 word_emb gather -> SBUF->SBUF DMA-transpose (XBAR) into the
  transposed residual layout; LN done ln_txp-style (stats via ones-matmuls)
  instead of the DVE-heavy natural-layout bn_stats path.
- Attention probs (exp scores) and augmented-V stored fp8 in DoubleRow pair
  layout -> ctx matmuls contract 256/pass (2 DR passes instead of 4 bf16).
- Wo in fp8 DoubleRow.  The ones-block of augmented V is 1/16 so the
  normalize multiply yields 16x ctx (good fp8 range); Wo is host-scaled by
  4 so the O-projection PSUM is 64x true.  The pre-LN residual carry is
  kept 64x-scaled (LN is scale-invariant; ln2/emb gammas are host-scaled
  by 64 for the carry output only), so no extra rescale op is needed.
- bv folded into bo host-side (bo64 = 64*(bo + bv@Wo)); rmask and the
  segment mask matrix G precomputed host-side.
- ln Square moved to gpsimd, mean scale to DVE, X8 fp8 pair casts and the
  ln1 bf16 copy to gpsimd (shorter DVE chains, fewer ACT table loads).
- Final h transposes + pool: single h_nat tile written by DMA-transpose
  from the bf16 LN outputs; pool accumulates c0's 4 token tiles while
  ln2(c1) finishes on DVE, then c1's.
"""

import os
import sys
from contextlib import ExitStack

import numpy as np
import ml_dtypes

for _p in ("/opt/trn_rl_repo", "/root/.axon_site/_ro/trn_rl_repo"):
    if os.path.isdir(_p) and _p not in sys.path:
        sys.path.append(_p)

import concourse.bass as bass
import concourse.tile as tile
from concourse import bacc, mybir
from concourse.bass_utils import run_bass_kernel_spmd

AF = mybir.ActivationFunctionType
ALU = mybir.AluOpType
F32 = mybir.dt.float32
BF16 = mybir.dt.bfloat16
FP8 = mybir.dt.float8e4
I32 = mybir.dt.int32
DR = mybir.MatmulPerfMode.DoubleRow
WS = 64.0   # fp8 weight scale for Wq/Wk/Wv

B, S, W = 8, 1024, 512
D, H, F, L, V = 768, 12, 3072, 4, 28996
CH = 512
P = 128
DT = D // P          # 6 d-tiles
FT = F // P          # 24 f-tiles
NH = H // 2          # 6 head pairs
KT = CH // P         # 4 key tiles per chunk
DH = D // H          # 64

_COLS = dict(bq=(0, 6), bk=(6, 6), bo=(12, 6), b1f=(18, 24), b2f=(42, 6),
             g1=(48, 6), b1=(54, 6), g2=(60, 6), b2=(66, 6),
             g2s=(72, 6), b2s=(78, 6))
SM_W = 84

N_CORES = 8
WS_HOST = 64.0
CARRY = 64.0   # scaled-residual factor (= 16 from den-ones * 4 from Wo scale)


def _col(sm, name, i):
    off, _n = _COLS[name]
    return sm[:, off + i:off + i + 1]


def build_kernel(ctx: ExitStack, tc: tile.TileContext, io: dict):
    nc = tc.nc

    consts = ctx.enter_context(tc.tile_pool(name="consts", bufs=1))
    big = ctx.enter_context(tc.tile_pool(name="big", bufs=1))
    psum = ctx.enter_context(tc.tile_pool(name="psum", bufs=1, space="PSUM"))

    # ---- constants / persistent ----
    ones_b = consts.tile([P, P], BF16, tag="onesb")
    nc.vector.memset(ones_b, 1.0)

    G_sb = big.tile([P, 8, W], BF16, tag="gsb")
    nc.sync.dma_start(out=G_sb, in_=io["G"])
    rmask = consts.tile([P, 4], F32, tag="rmask")
    nc.sync.dma_start(out=rmask, in_=io["rmask"])

    mask_sb = consts.tile([P, 8], F32, tag="masksb")
    nc.sync.dma_start(out=mask_sb, in_=io["mask128"])
    mb = consts.tile([P, 8], F32, tag="mb")
    nc.vector.tensor_scalar(mb, mask_sb, 10000.0, -10000.0,
                            op0=ALU.mult, op1=ALU.add)

    posT = consts.tile([P, DT, CH], BF16, tag="posT")
    nc.sync.dma_start(out=posT, in_=io["posT"])
    emb_sm = consts.tile([P, 24], F32, tag="embsm")
    nc.sync.dma_start(out=emb_sm, in_=io["emb_sm"])

    # final h in natural layout [tok128, (tile, d)]
    h_nat = big.tile([P, 8, D], BF16, tag="hnat")
    # augmented V, fp8 DR pair layout: [keys128, pair(2), head(12), 128]
    # cols 0:64 = 1/16 (denominator rows), 64:128 = V
    vaug = {c: [big.tile([P, 2, H, P], FP8, tag="vaug", bufs=4,
                         name=f"vaug{c}{jp}") for jp in range(2)]
            for c in (0, 1)}
    for c in (0, 1):
        for jp in range(2):
            nc.vector.memset(vaug[c][jp][:, :, :, 0:64], 1.0 / 16.0)

    work_ctx = ExitStack()
    work = work_ctx.enter_context(tc.tile_pool(name="work", bufs=1))

    def x8_from_xb(Xbn):
        """fp8 DR pair copies of 6 bf16 tiles, on gpsimd."""
        X8n = []
        for kp in range(DT // 2):
            x8t = work.tile([P, 2, CH], FP8, tag="x8", bufs=7, name="x8t")
            nc.gpsimd.tensor_copy(x8t[:, 0, :], Xbn[2 * kp])
            nc.gpsimd.tensor_copy(x8t[:, 1, :], Xbn[2 * kp + 1])
            X8n.append(x8t)
        return X8n

    def ln_txp(X1, gA, bA, gB, bB, want_x32=True, xb_same=False,
               pre_bf16=False):
        """LN over partition dim of transposed tiles.

        X1: 6 pre-LN tiles (fp32, or bf16 if pre_bf16).  Returns
        (X32new or None, Xbnew).  gA/bA: per-k [P,1] col APs for the fp32
        carry output; gB/bB for the bf16 GEMM-input output.  xb_same:
        carry and bf16 output share values -> bf16 copy made on gpsimd.
        """
        ps1 = psum.tile([P, CH], F32, tag="mm", bufs=3, name="lnps1")
        ps2 = psum.tile([P, CH], F32, tag="mm", bufs=3, name="lnps2")
        for k in range(DT):
            if pre_bf16:
                xb16 = X1[k]
            else:
                xb16 = work.tile([P, CH], BF16, tag="xb16p", bufs=2,
                                 name="lnxb16")
                nc.vector.tensor_copy(xb16, X1[k])
            sq = work.tile([P, CH], BF16, tag="sq", bufs=2, name="lnsq")
            nc.gpsimd.tensor_mul(sq, xb16, xb16)
            nc.tensor.matmul(ps1, ones_b, xb16,
                             start=(k == 0), stop=(k == DT - 1))
            nc.tensor.matmul(ps2, ones_b, sq,
                             start=(k == 0), stop=(k == DT - 1))
        mean = work.tile([P, CH], BF16, tag="stat", bufs=3, name="lnmean")
        nc.vector.tensor_scalar(mean, ps1, 1.0 / D, None, op0=ALU.mult)
        m2 = work.tile([P, CH], F32, tag="statf", bufs=2, name="lnm2")
        nc.vector.tensor_mul(m2, mean, mean)
        var = work.tile([P, CH], F32, tag="statf", bufs=2, name="lnvar")
        nc.vector.scalar_tensor_tensor(var, ps2, 1.0 / D, m2,
                                       op0=ALU.mult, op1=ALU.subtract)
        rv = work.tile([P, CH], F32, tag="statf", bufs=2, name="lnrv")
        nc.vector.reciprocal_approx_fast(rv, var)
        istd = work.tile([P, CH], BF16, tag="stat", bufs=3, name="lnistd")
        nc.scalar.activation(istd, rv, AF.Sqrt)
        X32n, Xbn = [], []
        for k in range(DT):
            t = work.tile([P, CH], F32, tag="lnt", bufs=2, name="lnt")
            nc.vector.tensor_sub(t, X1[k], mean)
            nc.vector.tensor_mul(t, t, istd)
            xbn = work.tile([P, CH], BF16, tag="xb", bufs=13, name="lnxb")
            if want_x32:
                x32 = work.tile([P, CH], F32, tag="x32", bufs=12,
                                name="lnx32")
                nc.vector.tensor_scalar(x32, t, gA(k), bA(k),
                                        op0=ALU.mult, op1=ALU.add)
                X32n.append(x32)
                if xb_same:
                    nc.gpsimd.tensor_copy(xbn, x32)
                else:
                    nc.vector.tensor_scalar(xbn, t, gB(k), bB(k),
                                            op0=ALU.mult, op1=ALU.add)
            else:
                nc.vector.tensor_scalar(xbn, t, gB(k), bB(k),
                                        op0=ALU.mult, op1=ALU.add)
            Xbn.append(xbn)
        return (X32n if want_x32 else None), Xbn

    # ================= embedding =================
    def embed_chunk(c):
        Xp = work.tile([P, DT, CH], BF16, tag="xe", bufs=2, name="xp")
        for tt in range(KT):
            ids_sb = work.tile([P, 1], I32, tag="ids", bufs=2, name="idssb")
            nc.sync.dma_start(out=ids_sb, in_=io["ids"][c * 4 + tt])
            eg = work.tile([P, D], BF16, tag="embg", bufs=2, name="embg")
            nc.gpsimd.indirect_dma_start(
                out=eg, out_offset=None, in_=io["word_emb"][:],
                in_offset=bass.IndirectOffsetOnAxis(ap=ids_sb[:, :1], axis=0))
            nc.sync.dma_start_transpose(Xp[:, :, tt * P:(tt + 1) * P], eg)
        Xe = work.tile([P, DT, CH], BF16, tag="xe", bufs=2, name="xe")
        nc.vector.tensor_add(Xe, Xp, posT)
        X32n, Xbn = ln_txp(
            [Xe[:, k, :] for k in range(DT)],
            gA=lambda k: emb_sm[:, 12 + k:13 + k],
            bA=lambda k: emb_sm[:, 18 + k:19 + k],
            gB=lambda k: emb_sm[:, 0 + k:1 + k],
            bB=lambda k: emb_sm[:, 6 + k:7 + k],
            pre_bf16=True)
        return X32n, Xbn, x8_from_xb(Xbn)

    e0 = embed_chunk(0)
    e1 = embed_chunk(1)
    X32 = {0: e0[0], 1: e1[0]}
    Xb = {0: e0[1], 1: e1[1]}
    X8 = {0: e0[2], 1: e1[2]}

    # ================= encoder layers =================
    pending_ln2 = None
    for l in range(L):
        last = (l == L - 1)
        sm = work.tile([P, SM_W], F32, tag="smalls", bufs=2, name="smalls")
        nc.sync.dma_start(out=sm, in_=io["smalls"][l])

        QT = {0: [None] * DT, 1: [None] * DT}
        KTt = {0: [None] * DT, 1: [None] * DT}
        cxp = {c: [work.tile([P, 2, CH], FP8, tag="cx8", bufs=7,
                             name=f"cxp{c}{t}") for t in range(3)]
               for c in (0, 1)}

        def qk_block(c, wkey, bn, dst, m):
            wsl = work.tile([P, DT, P], FP8, tag="wqk", bufs=6, name="wqksl")
            nc.sync.dma_start(out=wsl, in_=io[wkey][l, m])
            ps = psum.tile([P, CH], F32, tag="mm", bufs=3, name="qkps")
            for kp in range(DT // 2):
                nc.tensor.matmul(ps, wsl[:, 2 * kp:2 * kp + 2, :], X8[c][kp],
                                 start=(kp == 0), stop=(kp == DT // 2 - 1),
                                 perf_mode=DR)
            o = work.tile([P, CH], BF16, tag="qk", bufs=18, name="qkt")
            nc.vector.tensor_scalar(o, ps, 1.0 / WS, _col(sm, bn, m),
                                    op0=ALU.mult, op1=ALU.add)
            dst[c][m] = o

        def v_block(c, nn):
            wvs = []
            for kp in range(DT // 2):
                wv = work.tile([P, 2, 384], FP8, tag="wv", bufs=3, name="wvsl")
                nc.sync.dma_start(out=wv, in_=io["Wv"][l, kp, nn])
                wvs.append(wv)
            for mt in range(KT):
                ps = psum.tile([P, 384], F32, tag="mm", bufs=3, name="vps")
                for kp in range(DT // 2):
                    nc.tensor.matmul(ps, X8[c][kp][:, :, mt * P:(mt + 1) * P],
                                     wvs[kp], start=(kp == 0),
                                     stop=(kp == DT // 2 - 1), perf_mode=DR)
                nc.vector.tensor_scalar(
                    vaug[c][mt // 2][:, mt % 2, 6 * nn:6 * nn + 6, 64:128],
                    ps.rearrange("p (h d) -> p h d", h=6), 1.0 / WS, None,
                    op0=ALU.mult)

        Et = {}

        def scores(c, p):
            Et[(c, p)] = {hh: [work.tile([P, 2, CH], FP8, tag="e", bufs=11,
                                         name="etile") for _ in range(2)]
                          for hh in (0, 1)}
            for jk in range(KT):
                for hh in (0, 1):
                    lo = hh * 64
                    ps = psum.tile([P, CH], F32, tag="sc", bufs=3,
                                   name="scps")
                    nc.tensor.matmul(
                        ps, KTt[c][p][lo:lo + 64, jk * P:(jk + 1) * P],
                        QT[c][p][lo:lo + 64, :], start=True, stop=True)
                    nc.scalar.activation(
                        Et[(c, p)][hh][jk // 2][:, jk % 2, :], ps, AF.Exp,
                        scale=0.125,
                        bias=mb[:, c * 4 + jk: c * 4 + jk + 1])

        def augctx(c, p):
            for hh in (0, 1):
                h = 2 * p + hh
                pch = psum.tile([P, CH], F32, tag="cx", bufs=2, name="augps")
                for jp in range(2):
                    nc.tensor.matmul(pch, vaug[c][jp][:, :, h, :],
                                     Et[(c, p)][hh][jp],
                                     start=(jp == 0), stop=(jp == 1),
                                     perf_mode=DR)
                rec = work.tile([64, CH], F32, tag="rd", bufs=2, name="recd")
                nc.vector.reciprocal_approx_fast(rec, pch[0:64, :])
                nc.vector.tensor_mul(
                    cxp[c][p // 2][hh * 64:(hh + 1) * 64, p % 2, :],
                    pch[64:128, :], rec)
            del Et[(c, p)]

        X1 = {0: [None] * DT, 1: [None] * DT}

        def o_block(c, m):
            wsl = work.tile([P, 3, 2, P], FP8, tag="wo8", bufs=6, name="wosl")
            nc.sync.dma_start(out=wsl, in_=io["Wo8"][l, m])
            ps = psum.tile([P, CH], F32, tag="mm", bufs=3, name="ops")
            for kp in range(3):
                nc.tensor.matmul(ps, wsl[:, kp, :, :], cxp[c][kp],
                                 start=(kp == 0), stop=(kp == 2),
                                 perf_mode=DR)
            xp = work.tile([P, CH], F32, tag="pre", bufs=7, name="x1pre")
            nc.vector.scalar_tensor_tensor(xp, ps, _col(sm, "bo", m),
                                           X32[c][m], op0=ALU.add,
                                           op1=ALU.add)
            X1[c][m] = xp

        def ln1(c):
            x32n, xbn = ln_txp(X1[c],
                               gA=lambda k: _col(sm, "g1", k),
                               bA=lambda k: _col(sm, "b1", k),
                               gB=lambda k: _col(sm, "g1", k),
                               bB=lambda k: _col(sm, "b1", k),
                               xb_same=True)
            X32[c], Xb[c] = x32n, xbn

        # ---- phase A: QKV(c0); deferred ln2(c1) hidden behind Q(c0) ----
        for m in range(DT):
            qk_block(0, "Wq", "bq", QT, m)
        if pending_ln2 is not None:
            pending_ln2()
            pending_ln2 = None
        for m in range(DT):
            qk_block(0, "Wk", "bk", KTt, m)
        for nn in range(2):
            v_block(0, nn)

        # ---- phase B: attn(c0) zippered with Q/K(c1) ----
        fillB = ([lambda m=m: qk_block(1, "Wq", "bq", QT, m)
                  for m in range(DT)]
                 + [lambda m=m: qk_block(1, "Wk", "bk", KTt, m)
                    for m in range(DT)])
        fi = 0
        for p in range(NH):
            scores(0, p)
            for _ in range(2):
                if fi < len(fillB):
                    fillB[fi]()
                    fi += 1
            if p >= 1:
                augctx(0, p - 1)
        while fi < len(fillB):
            fillB[fi]()
            fi += 1
        augctx(0, NH - 1)

        # ---- phase C: attn(c1) zippered with V(c1) then O(c0) ----
        fillC = ([lambda nn=nn: v_block(1, nn) for nn in range(2)]
                 + [lambda m=m: o_block(0, m) for m in range(DT)])
        fi = 0
        for p in range(NH):
            scores(1, p)
            if fi < len(fillC):
                fillC[fi]()
                fi += 1
            if p >= 2:
                augctx(1, p - 2)
        while fi < len(fillC):
            fillC[fi]()
            fi += 1
        augctx(1, NH - 2)
        augctx(1, NH - 1)

        o_block(1, 0)
        o_block(1, 1)
        ln1(0)
        for m in range(2, DT):
            o_block(1, m)
        ln1(1)

        # ---- phase D: FFN + LN2 per chunk; FFN2 k-outer ----
        for c in (0, 1):
            facc = [psum.tile([P, CH], F32, tag=t, bufs=b, name=f"f2acc{m}")
                    for m, (t, b) in enumerate(
                        (("sc", 3), ("sc", 3), ("cx", 2),
                         ("cx", 2), ("mm", 3), ("mm", 3)))]
            H1 = []
            w2_sb = {}

            def ffn2_group(k):
                for m in range(DT):
                    nc.tensor.matmul(facc[m], w2_sb[k % 4][:, m * P:(m + 1) * P],
                                     H1[k], start=(k == 0), stop=(k == FT - 1))

            for mg in range(DT):
                w1_sb = work.tile([P, DT, CH], BF16, tag="w1", bufs=2,
                                  name="w1sb")
                nc.sync.dma_start(out=w1_sb, in_=io["W1"][l, mg])
                for mm2 in range(4):
                    k = mg * 4 + mm2
                    w2_sb[k % 4] = work.tile([P, D], BF16, tag="w2", bufs=3,
                                             name="w2sb")
                    nc.sync.dma_start(out=w2_sb[k % 4], in_=io["W2"][l, k])
                    ps = psum.tile([P, CH], F32, tag="sc", bufs=3, name="f1ps")
                    for kk in range(DT):
                        nc.tensor.matmul(
                            ps, w1_sb[:, kk, mm2 * P:(mm2 + 1) * P],
                            Xb[c][kk], start=(kk == 0), stop=(kk == DT - 1))
                    h1t = work.tile([P, CH], BF16, tag="h1", bufs=6, name="h1t")
                    nc.scalar.activation(h1t, ps, AF.Gelu,
                                         bias=_col(sm, "b1f", k))
                    H1.append(h1t)
                    if k > 0:
                        ffn2_group(k - 1)
            ffn2_group(FT - 1)
            X2 = []
            for m in range(DT):
                xp = work.tile([P, CH], F32, tag="pre", bufs=7, name="x2pre")
                nc.vector.scalar_tensor_tensor(xp, facc[m], _col(sm, "b2f", m),
                                               X32[c][m], op0=ALU.add,
                                               op1=ALU.add)
                X2.append(xp)

            def do_ln2(X2c, smc, cc, is_last):
                x32n, xbn = ln_txp(
                    X2c,
                    gA=lambda k: _col(smc, "g2s", k),
                    bA=lambda k: _col(smc, "b2s", k),
                    gB=lambda k: _col(smc, "g2", k),
                    bB=lambda k: _col(smc, "b2", k),
                    want_x32=not is_last)
                if not is_last:
                    X32[cc] = x32n
                Xb[cc] = xbn
                if not is_last:
                    X8[cc] = x8_from_xb(xbn)

            if c == 0:
                do_ln2(X2, sm, 0, last)
                if last:
                    # c0 final transposes + first half of pooling overlap
                    # the c1 FFN/LN tail below
                    for k in range(DT):
                        nc.sync.dma_start_transpose(
                            h_nat[:, 0:4, k * P:(k + 1) * P], Xb[0][k])
            else:
                if last:
                    do_ln2(X2, sm, 1, True)
                else:
                    def make_pending(X2c, smc):
                        def go():
                            do_ln2(X2c, smc, 1, False)
                        return go
                    pending_ln2 = make_pending(X2, sm)

    # ================= segment mean-pool =================
    # pool psums: all 8 banks
    pps = [psum.tile([P, 384], F32, tag=t, bufs=b, name=f"poolps{i}")
           for i, (t, b) in enumerate(
               (("sc", 3), ("sc", 3), ("sc", 3), ("mm", 3),
                ("mm", 3), ("mm", 3), ("cx", 2), ("cx", 2)))]

    def pool_half(trange, final):
        for w in range(4):
            for dn in range(2):
                acc = pps[w * 2 + dn]
                for t in trange:
                    nc.tensor.matmul(acc, G_sb[:, t, w * P:(w + 1) * P],
                                     h_nat[:, t, dn * 384:(dn + 1) * 384],
                                     start=(t == 0), stop=(final and t == trange[-1]))

    # c0 half runs while ln2(c1) apply chain finishes on DVE
    pool_half(range(4), False)
    for k in range(DT):
        nc.sync.dma_start_transpose(h_nat[:, 4:8, k * P:(k + 1) * P], Xb[1][k])
    pool_half(range(4, 8), True)
    for w in range(4):
        for dn in range(2):
            o = work.tile([P, 384], F32, tag="poolo", bufs=2, name="poolo")
            nc.scalar.activation(o, pps[w * 2 + dn], AF.Copy,
                                 scale=rmask[:, w:w + 1])
            nc.sync.dma_start(
                out=io["out"][w * P:(w + 1) * P, dn * 384:(dn + 1) * 384],
                in_=o)


def build_program():
    nc = bacc.Bacc("TRN2", target_bir_lowering=False, debug=False,
                   num_devices=N_CORES)
    io = {}

    def inp(name, shape, dt):
        io[name] = nc.dram_tensor(name, list(shape), dt, kind="ExternalInput").ap()

    inp("ids", (8, P, 1), I32)
    inp("mask128", (P, 8), F32)
    inp("G", (P, 8, W), BF16)
    inp("rmask", (P, 4), F32)
    inp("word_emb", (V, D), BF16)
    inp("posT", (P, DT, CH), BF16)
    inp("emb_sm", (P, 24), F32)
    inp("smalls", (L, P, SM_W), F32)
    inp("Wq", (L, DT, P, DT, P), FP8)
    inp("Wk", (L, DT, P, DT, P), FP8)
    inp("Wo8", (L, DT, P, 3, 2, P), FP8)
    inp("Wv", (L, DT // 2, 2, P, 2, 384), FP8)
    inp("W1", (L, DT, P, DT, CH), BF16)
    inp("W2", (L, FT, P, D), BF16)
    io["out"] = nc.dram_tensor("out", [W, D], F32, kind="ExternalOutput").ap()

    with tile.TileContext(nc) as tc:
        with ExitStack() as ctx:
            build_kernel(ctx, tc, io)
    nc.compile()
    return nc


_NC_CACHE = None


def _get_program():
    global _NC_CACHE
    if _NC_CACHE is None:
        _NC_CACHE = build_program()
    return _NC_CACHE


def make_in_maps(inputs):
    """Host-side prep: shard per batch row, reshape/cast into device layouts."""
    bf = ml_dtypes.bfloat16
    f8 = ml_dtypes.float8_e4m3
    x_bert = np.asarray(inputs["x_bert"])
    x_mask_tok = np.asarray(inputs["x_bert_mask"], dtype=np.float32)
    off = np.asarray(inputs["x_bert_offset"])
    xm = np.asarray(inputs["x_mask"])
    word_emb = np.ascontiguousarray(
        np.asarray(inputs["word_emb"], np.float32).astype(bf))
    pos_type = np.asarray(inputs["pos_emb"], np.float32) + \
        np.asarray(inputs["type_emb"], np.float32)[0][None, :]
    # posT[p, k, j] = pos_type[j, k*128 + p]
    posT = np.ascontiguousarray(
        pos_type.T.reshape(DT, P, CH).transpose(1, 0, 2).astype(bf))

    emb_g = np.asarray(inputs["emb_g"], np.float32)
    emb_b = np.asarray(inputs["emb_b"], np.float32)
    emb_sm = np.zeros((P, 24), np.float32)
    emb_sm[:, 0:6] = emb_g.reshape(DT, P).T
    emb_sm[:, 6:12] = emb_b.reshape(DT, P).T
    emb_sm[:, 12:18] = emb_sm[:, 0:6] * CARRY
    emb_sm[:, 18:24] = emb_sm[:, 6:12] * CARRY

    wo_f = np.asarray(inputs["Wo"], np.float32)
    bo_f = np.asarray(inputs["bo"], np.float32)
    bv_f = np.asarray(inputs["bv"], np.float32)
    # fold bv through Wo into the o bias; 64x for the scaled carry
    bo64 = CARRY * (bo_f + np.einsum('ld,lde->le', bv_f, wo_f))

    smalls = np.zeros((L, P, SM_W), np.float32)
    for nm, arr in (("bq", np.asarray(inputs["bq"], np.float32)),
                    ("bk", np.asarray(inputs["bk"], np.float32)),
                    ("bo", bo64),
                    ("b1f", np.asarray(inputs["b1f"], np.float32)),
                    ("b2f", np.asarray(inputs["b2f"], np.float32)),
                    ("g1", np.asarray(inputs["ln1_g"], np.float32)),
                    ("b1", np.asarray(inputs["ln1_b"], np.float32)),
                    ("g2", np.asarray(inputs["ln2_g"], np.float32)),
                    ("b2", np.asarray(inputs["ln2_b"], np.float32))):
        offc, n = _COLS[nm]
        smalls[:, :, offc:offc + n] = arr.reshape(L, n, P).transpose(0, 2, 1)
    offc, n = _COLS["g2s"]
    smalls[:, :, offc:offc + n] = smalls[:, :, _COLS["g2"][0]:_COLS["g2"][0] + n] * CARRY
    offc, n = _COLS["b2s"]
    smalls[:, :, offc:offc + n] = smalls[:, :, _COLS["b2"][0]:_COLS["b2"][0] + n] * CARRY

    wts = {}
    for k in ("Wq", "Wk"):
        w = np.asarray(inputs[k], np.float32) * WS_HOST    # [L, D, D]
        wts[k] = np.ascontiguousarray(
            w.reshape(L, DT, P, DT, P).transpose(0, 3, 2, 1, 4).astype(f8))
    # Wo8[l, m, row, kp, r, col] = 4*Wo[l, (2kp+r)*128+row, m*128+col]
    wo = wo_f * (CARRY / 16.0)
    wts["Wo8"] = np.ascontiguousarray(
        wo.reshape(L, 3, 2, P, DT, P).transpose(0, 4, 3, 1, 2, 5).astype(f8))
    wv = np.asarray(inputs["Wv"], np.float32) * WS_HOST
    wts["Wv"] = np.ascontiguousarray(
        wv.reshape(L, DT // 2, 2, P, 2, 384).transpose(0, 1, 4, 3, 2, 5)
        .astype(f8))
    w1 = np.asarray(inputs["W1"], np.float32).astype(bf)
    wts["W1"] = np.ascontiguousarray(
        w1.reshape(L, DT, P, DT, CH).transpose(0, 3, 2, 1, 4))   # [L,mg,P,k,CH]
    w2 = np.asarray(inputs["W2"], np.float32).astype(bf)
    wts["W2"] = np.ascontiguousarray(w2.reshape(L, FT, P, D))    # [L,k,P,D]

    tok_idx = np.arange(S, dtype=np.int64)

    in_maps = []
    for b in range(N_CORES):
        ids = np.ascontiguousarray(
            x_bert[b].astype(np.int32).reshape(8, P, 1))
        mask128 = np.ascontiguousarray(
            x_mask_tok[b].reshape(8, P).T.astype(np.float32))
        st = off[b, :, 0]
        ed = off[b, :, 1]
        # G[p, t, w] = 1 if st_w <= t*128+p < ed_w
        gm = ((st[None, :] <= tok_idx[:, None])
              & (tok_idx[:, None] < ed[None, :]))          # [S, W]
        G = np.ascontiguousarray(
            gm.reshape(8, P, W).transpose(1, 0, 2).astype(bf))
        cnt = np.maximum((ed - st).astype(np.float32), 1.0)
        valid = (xm[b] != 0) & (st < ed)
        rm = np.where(valid, 1.0 / cnt, 0.0).astype(np.float32)
        m = {
            "ids": ids,
            "mask128": mask128,
            "G": G,
            "rmask": np.ascontiguousarray(rm.reshape(4, P).T),
            "word_emb": word_emb,
            "posT": posT,
            "emb_sm": emb_sm,
            "smalls": smalls,
        }
        m.update(wts)
        in_maps.append(m)
    return in_maps


def kernel(**inputs):
    nc = _get_program()
    in_maps = make_in_maps(inputs)
    res = run_bass_kernel_spmd(nc, in_maps, list(range(N_CORES)))
    return np.stack([res.results[b]["out"] for b in range(N_CORES)])
